# revision 1
# baseline (speedup 1.0000x reference)
"""DGRec kernel for 8 TRN2 NeuronCores (Bass/Tile).

Strategy:
  - Host: index-only prep + table row-selection sharding. Live-session pruning
    (sessions never referenced by src0/idx0/cur_sidx are dead), sessions sorted
    by lens desc and dealt round-robin so every core shares one static
    shrinking-prefix LSTM schedule. Per-core compact item vocab; per-core user
    rows in session order; item vocab shard for logits.
  - Device: renorm tables (bf16), dma_gather transpose-mode loads tokens
    directly into [D, sessions] column layout, LSTM gates via 4-slot PSUM
    matmuls + ACT sigmoid/tanh (per-gate bias), bf16 DVE state updates,
    GAT via one-hot matmuls (edge softmax, segment sums), two AllGathers,
    vocab-sharded logits matmul.
"""
import sys
sys.path.insert(0, '/opt/trn_rl_repo')

import numpy as np


class _PhaseDone(Exception):
    pass


NCORES = 8
D = 128
T = 20
N0 = 25600
N1 = 2560
N2 = 512
E1 = 5120
NI = 50000
CURB = 128          # cur block rows per core (slot CURB-1 reserved zero row)
DST0_PER_CORE = N1 // NCORES  # 320
B1 = 384            # feat1 block rows per core (320 padded to 384)


def _rup(x, m):
    return (int(x) + m - 1) // m * m


def _wrap16(idx):
    """[n] int -> [128, n//16] int16: idx i at [i%16, i//16], replicated x8."""
    idx = np.asarray(idx, np.int16)
    n = len(idx)
    assert n % 16 == 0
    a = idx.reshape(n // 16, 16).T  # [16, n//16]
    return np.tile(a, (8, 1))


def host_prep(inputs):
    lens = np.asarray(inputs['lens']).astype(np.int64)
    seqs = np.asarray(inputs['padded_seqs']).astype(np.int64)
    uids = np.asarray(inputs['uids']).astype(np.int64)
    cur_sidx = np.asarray(inputs['cur_sidx']).astype(np.int64)
    src0 = np.asarray(inputs['src0']).astype(np.int64)
    dst0 = np.asarray(inputs['dst0']).astype(np.int64)
    idx0 = np.asarray(inputs['idx0']).astype(np.int64)
    src1 = np.asarray(inputs['src1']).astype(np.int64)
    dst1 = np.asarray(inputs['dst1']).astype(np.int64)
    idx1 = np.asarray(inputs['idx1']).astype(np.int64)
    user_emb = np.ascontiguousarray(np.asarray(inputs['user_emb'], np.float32))
    item_emb = np.ascontiguousarray(np.asarray(inputs['item_emb'], np.float32))

    # ---- live sessions, sorted by len desc, per-len-group padded to mult 8
    live_mask = np.zeros(N0, bool)
    live_mask[src0] = True
    live_mask[idx0] = True
    live_mask[cur_sidx] = True
    live = np.where(live_mask)[0]
    order = live[np.argsort(-lens[live], kind='stable')]
    lens_live = lens[order]
    parts = []
    grp_ceil = {}
    for L in range(T, 0, -1):
        grp = order[lens_live == L]
        pad = (-len(grp)) % NCORES
        parts.append(grp)
        grp_ceil[L] = (len(grp) + pad) // NCORES
        if pad:
            parts.append(np.full(pad, -1, np.int64))
    order_p = np.concatenate(parts)
    percore = len(order_p) // NCORES
    NL = _rup(percore, 128)
    extra = NL * NCORES - len(order_p)
    order_p = np.concatenate([order_p, np.full(extra, -1, np.int64)])
    core_sessions = [order_p[k::NCORES] for k in range(NCORES)]
    act = [sum(grp_ceil[L] for L in range(t + 1, T + 1)) for t in range(T)]
    act = [min(a, NL) for a in act]

    # ---- FTAB layout: per-core block [NL feat rows][CURB cur rows]
    BLK = NL + CURB
    ZROW = NL + CURB - 1  # core 0 block, slot CURB-1: reserved all-zero row
    sess2pos = np.full(N0, -1, np.int64)
    core_local = []       # per core: {session: local index}
    JU_ = NL // 128
    for k in range(NCORES):
        sess = core_sessions[k]
        real = sess >= 0
        li = np.where(real)[0]
        sess2pos[sess[real]] = k * BLK + (li % 128) * JU_ + li // 128
        core_local.append({int(s): i for i, s in enumerate(sess) if s >= 0})
    # cur block: per core, unique local cur sessions -> slots 0..cnt-1
    cur_pos = {}
    cur_slot_local = [[] for _ in range(NCORES)]
    for s in np.unique(cur_sidx):
        owner = -1
        for k in range(NCORES):
            if int(s) in core_local[k]:
                owner = k
                break
        assert owner >= 0
        slot = len(cur_slot_local[owner])
        assert slot < 112
        cur_pos[int(s)] = owner * BLK + NL + slot
        cur_slot_local[owner].append(core_local[owner][int(s)])
    # remap: sessions in cur_sidx read hn (feat.at[cur].set(cur))
    for s in np.unique(cur_sidx):
        sess2pos[s] = cur_pos[int(s)]

    # ---- per-core compact vocab + token idx per step (pass 1: sizes)
    vocab_sizes = []
    for k in range(NCORES):
        sess = core_sessions[k]
        smax = np.maximum(sess, 0)
        toks = np.where(sess[None, :] >= 0, seqs[smax].T, 0)
        used = np.concatenate([toks[t, :act[t]] for t in range(T) if act[t] > 0])
        vocab_sizes.append(len(np.unique(np.concatenate([[0], used]))))
    VC0 = _rup(max(vocab_sizes), 128)

    core_vocab = []
    core_tok = []   # [T] arrays of compact ids, each length rup(act[t],128)
    for k in range(NCORES):
        sess = core_sessions[k]
        smax = np.maximum(sess, 0)
        toks = np.where(sess[None, :] >= 0, seqs[smax].T, 0)  # [T, NL]
        used = np.concatenate([toks[t, :act[t]] for t in range(T) if act[t] > 0])
        vocab = np.unique(np.concatenate([[0], used]))
        lut = np.full(NI + 1, -1, np.int64)
        # table rows are written contiguously from [128, J, 128] staging:
        # staging slot (p, j) = vocab[128*j+p] lands at DRAM row p*J + j
        JI_ = VC0 // 128
        ar = np.arange(len(vocab))
        lut[vocab] = (ar % 128) * JI_ + ar // 128
        per_step = []
        for t in range(T):
            a128 = _rup(max(act[t], 1), 128)
            ct = np.zeros(a128, np.int64)
            ct[:act[t]] = lut[toks[t, :act[t]]]
            per_step.append(ct)
        core_vocab.append(vocab)
        core_tok.append(per_step)
    VC = VC0
    assert VC < 32000

    # ---- GAT0: edges sharded by dst range, sorted by dst
    g0 = []
    for k in range(NCORES):
        lo, hi = k * DST0_PER_CORE, (k + 1) * DST0_PER_CORE
        e = np.where((dst0 >= lo) & (dst0 < hi))[0]
        e = e[np.argsort(dst0[e], kind='stable')]
        g0.append(e)
    E0C = _rup(max(len(e) for e in g0), 128)
    NCH0 = E0C // 128

    # ---- GAT1 (redundant on all cores): edges sorted by dst
    e1 = np.argsort(dst1, kind='stable')
    assert E1 % 128 == 0
    NCH1 = E1 // 128

    def f1pos(node):
        node = np.asarray(node)
        loc = node % DST0_PER_CORE
        return (node // DST0_PER_CORE) * B1 + (loc % 128) * 3 + loc // 128

    g1_pay = f1pos(src1[e1])
    g1_fd = f1pos(idx1[dst1[e1]])
    g1_dst = dst1[e1]
    g1_res = f1pos(idx1)  # [512]

    cur_idx = np.array([cur_pos[int(s)] for s in cur_sidx], np.int64)

    LSH = NI // NCORES          # 6250
    LSHP = _rup(LSH, 128)       # 6272

    meta = dict(NL=NL, BLK=BLK, VC=VC, E0C=E0C, NCH0=NCH0, NCH1=NCH1,
                LSH=LSH, LSHP=LSHP, act=act, ZROW=ZROW)

    # ---- IDX16 buffer layout (columns of 16-wrapped idx); same offsets all cores
    seg_off = {}
    _w = [0]

    def add_seg(name, n):
        seg_off[name] = _w[0]
        _w[0] += n // 16

    for t in range(T):
        add_seg(f'tok{t}', _rup(max(act[t], 1), 128))
    add_seg('g0pay', E0C)
    add_seg('g0fd', E0C)
    add_seg('g0res', B1)
    add_seg('g1pay', E1)
    add_seg('g1fd', E1)
    add_seg('g1res', 512)
    add_seg('cur', 512)
    add_seg('lsh', LSHP)
    add_seg('hnrow', 112)
    W16 = _w[0]
    meta['seg_off'] = seg_off
    meta['W16'] = W16

    in_maps = []
    for k in range(NCORES):
        sess = core_sessions[k]
        smax = np.maximum(sess, 0)
        vocab = core_vocab[k]

        idx16 = np.zeros((128, W16), np.int16)

        def put(name, vals):
            v = np.asarray(vals, np.int64)
            assert v.min() >= -1 and v.max() < 32767, (name, v.min(), v.max())
            o = seg_off[name]
            w = _wrap16(v.astype(np.int16))
            idx16[:, o:o + w.shape[1]] = w

        for t in range(T):
            put(f'tok{t}', core_tok[k][t])
        e = g0[k]
        pay = np.full(E0C, ZROW, np.int64)
        fde = np.full(E0C, ZROW, np.int64)
        dstl = np.full(E0C, -1, np.int64)
        pay[:len(e)] = sess2pos[src0[e]]
        fde[:len(e)] = sess2pos[idx0[dst0[e]]]
        dstl[:len(e)] = dst0[e] - k * DST0_PER_CORE
        put('g0pay', pay)
        put('g0fd', fde)
        res0 = np.full(B1, ZROW, np.int64)
        res0[:DST0_PER_CORE] = sess2pos[
            idx0[k * DST0_PER_CORE:(k + 1) * DST0_PER_CORE]]
        put('g0res', res0)
        put('g1pay', g1_pay)
        put('g1fd', g1_fd)
        put('g1res', g1_res)
        put('cur', cur_idx)
        lshi = np.zeros(LSHP, np.int64)
        ar = np.arange(LSH)
        lshi[:LSH] = (ar % 128) * (LSHP // 128) + ar // 128
        put('lsh', lshi)
        hnrow = np.zeros(112, np.int64)
        cs = np.asarray(cur_slot_local[k], np.int64)
        assert len(cs) <= 112
        if len(cs):
            hnrow[:len(cs)] = (cs % 128) * (NL // 128) + cs // 128
        put('hnrow', hnrow)

        # dst-local values for one-hot compare: edge (chunk c, partition p)
        dstl_all = np.full((128, NCH0 + NCH1), -1, np.int32)
        dstl_all[:, :NCH0] = dstl.reshape(NCH0, 128).T
        dstl_all[:, NCH0:] = g1_dst.reshape(NCH1, 128).T

        item_sub = np.zeros((VC, D), np.float32)
        item_sub[:len(vocab)] = item_emb[vocab]
        usr = np.zeros((NL, D), np.float32)
        usr[:] = user_emb[uids[smax]]
        usr[sess < 0] = 0.0
        ishard = np.zeros((LSHP, D), np.float32)
        ishard[:LSH] = item_emb[1 + k * LSH: 1 + (k + 1) * LSH]

        in_maps.append({
            'item_sub': item_sub,
            'usr': usr,
            'ishard': ishard,
            'WihT': np.ascontiguousarray(np.asarray(inputs['Wih'], np.float32).T),
            'WhhT': np.ascontiguousarray(np.asarray(inputs['Whh'], np.float32).T),
            'bih': np.asarray(inputs['bih'], np.float32),
            'bhh': np.asarray(inputs['bhh'], np.float32),
            'W1T': np.ascontiguousarray(np.asarray(inputs['W1'], np.float32).T),
            'W2T': np.ascontiguousarray(np.asarray(inputs['W2'], np.float32).T),
            'gW0T': np.ascontiguousarray(np.asarray(inputs['gW0'], np.float32).T),
            'gW1T': np.ascontiguousarray(np.asarray(inputs['gW1'], np.float32).T),
            'gb0': np.asarray(inputs['gb0'], np.float32),
            'gb1': np.asarray(inputs['gb1'], np.float32),
            'idx16': idx16,
            'dstl': dstl_all,
        })
    return in_maps, meta


# ============================ device program ============================

def build_program(meta):
    import os
    PHASE = int(os.environ.get('KPHASE', '9'))
    KSUB = int(os.environ.get('KSUB', '9'))
    KT = int(os.environ.get('KT', '99'))
    KCH = int(os.environ.get('KCH', '9'))
    import contextlib
    import concourse.bass as bass
    import concourse.mybir as mybir
    import concourse.tile as tile
    from concourse import bacc
    from concourse.masks import make_identity

    NL = meta['NL']
    BLK = meta['BLK']
    VC = meta['VC']
    NCH0 = meta['NCH0']
    NCH1 = meta['NCH1']
    LSH = meta['LSH']
    LSHP = meta['LSHP']
    act = meta['act']
    seg = meta['seg_off']
    W16 = meta['W16']
    JI = VC // 128
    JU = NL // 128
    JL = LSHP // 128
    FT = mybir.dt.float32
    BF = mybir.dt.bfloat16
    AF = mybir.ActivationFunctionType
    OP = mybir.AluOpType

    nc = bacc.Bacc("TRN2", target_bir_lowering=False, debug=False,
                   num_devices=NCORES)

    def param(name, shape, dt=FT):
        return nc.declare_dram_parameter(name, list(shape), dt, isOutput=False)

    item_sub = param('item_sub', [VC, D])
    usr = param('usr', [NL, D])
    ishard = param('ishard', [LSHP, D])
    WihT = param('WihT', [D, 512])
    WhhT = param('WhhT', [D, 512])
    bih = param('bih', [512])
    bhh = param('bhh', [512])
    W1T = param('W1T', [256, D])
    W2T = param('W2T', [256, D])
    gW0T = param('gW0T', [D, D])
    gW1T = param('gW1T', [D, D])
    gb0 = param('gb0', [D])
    gb1 = param('gb1', [D])
    idx16_p = param('idx16', [128, W16], mybir.dt.int16)
    dstl_p = param('dstl', [128, NCH0 + NCH1], mybir.dt.int32)
    out_p = nc.declare_dram_parameter('out', [N2, LSH], FT, isOutput=True)

    def rows_ap(handle_ap, j_count, base_elem=0):
        """view rows [128*j_count, D] of a DRAM tensor as [128, j, D], row=128j+p"""
        t = handle_ap if isinstance(handle_ap, bass.AP) else handle_ap[:]
        return bass.AP(tensor=t.tensor, offset=t.offset + base_elem,
                       ap=[[D, 128], [128 * D, j_count], [1, D]])

    def cont_ap(handle_ap, j_count, base_elem=0):
        """contiguous [128, j, D] view: slot (p, j) -> DRAM row p*j_count + j"""
        t = handle_ap if isinstance(handle_ap, bass.AP) else handle_ap[:]
        return bass.AP(tensor=t.tensor, offset=t.offset + base_elem,
                       ap=[[j_count * D, 128], [D, j_count], [1, D]])

    with tile.TileContext(nc) as tc:
        try:
            ctx = contextlib.ExitStack()
            ctx.__enter__()
            glob = ctx.enter_context(tc.tile_pool(name='glob', bufs=1))
            dram = ctx.enter_context(tc.tile_pool(name='dram', bufs=1,
                                                  space='DRAM'))

            ITBL = dram.tile([VC, D], BF)
            LSHARD = dram.tile([LSHP, D], BF)
            HNROWS = dram.tile([NL, D], BF)
            AGIN = dram.tile([BLK, D], BF)
            FTAB = dram.tile([NCORES * BLK, D], BF, addr_space='Shared')
            AG2IN = dram.tile([B1, D], BF)
            F1TAB = dram.tile([NCORES * B1, D], BF, addr_space='Shared')

            # ---- global constants / index tiles
            idx_sb = glob.tile([128, W16], mybir.dt.int16)
            nc.sync.dma_start(idx_sb[:], idx16_p[:])

            def seg_ap(name, n, off=0):
                o = seg[name] + off // 16
                return idx_sb[:, o:o + n // 16]

            GMAX = 512

            def gather_t(out_full, colbase, tab, name, n, queue=0):
                """transpose-mode gather of n idx (mult 128) from segment
                `name` into out_full[:, 0, colbase:colbase+n], split <=GMAX"""
                for o in range(0, n, GMAX):
                    w = min(GMAX, n - o)
                    nc.gpsimd.dma_gather(
                        out_ap=out_full[:, :, colbase + o:colbase + o + w],
                        in_ap=tab[:], idxs_ap=seg_ap(name, w, o),
                        num_idxs=w, num_idxs_reg=w, elem_size=D,
                        transpose=True, queue_num=queue)

            def gather_rows(out_tile, tab, name, n, queue=0):
                """non-transpose gather of n idx into [128, n//128, 128]"""
                for o in range(0, n, GMAX):
                    w = min(GMAX, n - o)
                    nc.gpsimd.dma_gather(
                        out_ap=out_tile[:, o // 128:(o + w) // 128, :],
                        in_ap=tab[:], idxs_ap=seg_ap(name, w, o),
                        num_idxs=w, num_idxs_reg=w, elem_size=D,
                        transpose=False, queue_num=queue)

            ident = glob.tile([128, 128], BF)
            make_identity(nc, ident[:])
            iota_i = glob.tile([128, 512], mybir.dt.int32)
            nc.gpsimd.iota(iota_i[:], pattern=[[1, 512]], base=0,
                           channel_multiplier=0)
            iotaf = glob.tile([128, 512], FT)
            nc.vector.tensor_copy(iotaf[:], iota_i[:])
            dstl_i = glob.tile([128, NCH0 + NCH1], mybir.dt.int32)
            nc.sync.dma_start(dstl_i[:], dstl_p[:])
            dstf = glob.tile([128, NCH0 + NCH1], FT)
            nc.vector.tensor_copy(dstf[:], dstl_i[:])
            ones1 = glob.tile([1, 128], FT)
            nc.vector.memset(ones1[:], 1.0)

            # ---- weights (cast to bf16 via gpsimd DMA)
            wih16 = glob.tile([128, 512], BF)
            nc.gpsimd.dma_start(wih16[:], WihT[:])
            whh16 = glob.tile([128, 512], BF)
            nc.gpsimd.dma_start(whh16[:], WhhT[:])
            w1_16 = glob.tile([128, 2, 128], BF)
            nc.gpsimd.dma_start(w1_16[:], rows_ap(W1T, 2))
            w2_16 = glob.tile([128, 2, 128], BF)
            nc.gpsimd.dma_start(w2_16[:], rows_ap(W2T, 2))
            gw0_16 = glob.tile([128, 128], BF)
            nc.gpsimd.dma_start(gw0_16[:], gW0T[:])
            gw1_16 = glob.tile([128, 128], BF)
            nc.gpsimd.dma_start(gw1_16[:], gW1T[:])
            gb0_sb = glob.tile([128, 1], FT)
            nc.sync.dma_start(gb0_sb[:], bass.AP(tensor=gb0, offset=0,
                                                 ap=[[1, 128], [1, 1]]))
            gb1_sb = glob.tile([128, 1], FT)
            nc.sync.dma_start(gb1_sb[:], bass.AP(tensor=gb1, offset=0,
                                                 ap=[[1, 128], [1, 1]]))
            bi_sb = glob.tile([128, 4], FT)
            nc.sync.dma_start(bi_sb[:], bass.AP(tensor=bih, offset=0,
                                                ap=[[1, 128], [128, 4]]))
            bh_sb = glob.tile([128, 4], FT)
            nc.sync.dma_start(bh_sb[:], bass.AP(tensor=bhh, offset=0,
                                                ap=[[1, 128], [128, 4]]))
            bias = glob.tile([128, 4], FT)
            nc.vector.tensor_add(bias[:], bi_sb[:], bh_sb[:])

            # ---- renorm: rows of src -> unit-clipped bf16 [128, J, 128]
            def renorm(pool, src, J, zero_row0=False, rows=None, sub=False):
                stg = pool.tile([128, J, 128], FT, tag='rn_stg')
                if rows is None or rows == 128 * J:
                    nc.sync.dma_start(stg[:], rows_ap(src, J))
                else:
                    jf = rows // 128
                    rem = rows - jf * 128
                    if jf:
                        nc.sync.dma_start(stg[:, :jf, :], rows_ap(src, jf))
                    if rem:
                        nc.vector.memset(stg[:, jf, :], 0.0)
                        s = src[:]
                        nc.sync.dma_start(
                            stg[:rem, jf, :],
                            bass.AP(tensor=s.tensor,
                                    offset=s.offset + jf * 128 * D,
                                    ap=[[D, rem], [1, D]]))
                if sub and KSUB < 3:
                    raise _PhaseDone()
                sumsq = pool.tile([128, J], FT, tag='rn_sumsq')
                sq16 = pool.tile([128, J, 128], BF, tag='rn_sq')
                nc.scalar.activation(out=sq16[:], in_=stg[:], func=AF.Square)
                nc.vector.tensor_reduce(out=sumsq[:], in_=sq16[:],
                                        axis=mybir.AxisListType.X, op=OP.add)
                if sub and KSUB < 4:
                    raise _PhaseDone()
                nrm = pool.tile([128, J], FT, tag='rn_nrm')
                nc.scalar.activation(out=nrm[:], in_=sumsq[:], func=AF.Sqrt)
                nc.vector.tensor_scalar_max(nrm[:], nrm[:], 1e-12)
                rcp = pool.tile([128, J], FT, tag='rn_rcp')
                nc.vector.reciprocal(rcp[:], nrm[:])
                nc.vector.tensor_scalar_min(rcp[:], rcp[:], 1.0)
                if sub and KSUB < 5:
                    raise _PhaseDone()
                out16 = pool.tile([128, J, 128], BF, tag='rn_out')
                J0 = (3 * J // 4) if J > 4 else J
                if J0:
                    r = rcp[:]
                    bc = bass.AP(tensor=r.tensor, offset=r.offset,
                                 ap=[r.ap[0], [r.ap[1][0], J0], [0, 128]])
                    nc.vector.tensor_tensor(out=out16[:, :J0, :],
                                            in0=stg[:, :J0, :], in1=bc,
                                            op=OP.mult)
                for j in range(J0, J):
                    nc.scalar.activation(out=out16[:, j, :], in_=stg[:, j, :],
                                         func=AF.Copy, scale=rcp[:, j:j + 1])
                if zero_row0:
                    nc.vector.memset(out16[0:1, 0, :], 0.0)
                return out16

            # ================= precompute: item table =================
            if KSUB < 2:
                raise _PhaseDone()
            with tc.tile_pool(name='pre_i', bufs=1) as pp:
                it16 = renorm(pp, item_sub, JI, zero_row0=True, sub=True)
                if KSUB < 6:
                    raise _PhaseDone()
                nc.sync.dma_start(cont_ap(ITBL, JI), it16[:])

            # ================= LSTM =================
            if PHASE < 2:
                raise _PhaseDone()
            hT = glob.tile([128, NL], BF)
            cT = glob.tile([128, NL], BF)
            nc.vector.memset(hT[:], 0.0)
            nc.vector.memset(cT[:], 0.0)

            A0 = _rup(act[0], 128)
            with (
                tc.tile_pool(name='lstm_x', bufs=3) as xp,
                tc.tile_pool(name='lstm_g', bufs=3) as sp,
                tc.tile_pool(name='lstm_ps', bufs=2, space='PSUM') as gp,
            ):
                for t in range(T):
                    if t >= KT:
                        break
                    a = act[t]
                    if a == 0:
                        break
                    a128 = _rup(a, 128)
                    xT = xp.tile([128, 1, A0], BF, tag='xT')
                    gather_t(xT, 0, ITBL, f'tok{t}', a128)
                    if KCH < 1:
                        continue
                    nch = (a + 511) // 512
                    for c in range(nch):
                        cs = c * 512
                        cw = min(512, a - cs)
                        ce = cs + cw
                        g4 = gp.tile([128, 4, 512], FT, tag='g4')
                        for g in range(4):
                            nc.tensor.matmul(
                                g4[:, g, :cw],
                                lhsT=wih16[:, g * 128:(g + 1) * 128],
                                rhs=xT[:, 0, cs:ce], start=True, stop=(t == 0))
                            if t > 0:
                                nc.tensor.matmul(
                                    g4[:, g, :cw],
                                    lhsT=whh16[:, g * 128:(g + 1) * 128],
                                    rhs=hT[:, cs:ce], start=False, stop=True)
                        if KCH < 2:
                            continue
                        sg = sp.tile([128, 4, 512], BF, tag='sg')
                        for g, fn in ((0, AF.Sigmoid), (1, AF.Sigmoid),
                                      (2, AF.Tanh), (3, AF.Sigmoid)):
                            nc.scalar.activation(out=sg[:, g, :cw],
                                                 in_=g4[:, g, :cw], func=fn,
                                                 bias=bias[:, g:g + 1])
                        if KCH < 3:
                            continue
                        if t > 0:
                            tmp = sp.tile([128, 512], BF, tag='tmp')
                            nc.vector.tensor_mul(tmp[:, :cw], sg[:, 0, :cw],
                                                 sg[:, 2, :cw])
                            nc.vector.tensor_mul(cT[:, cs:ce], cT[:, cs:ce],
                                                 sg[:, 1, :cw])
                            nc.vector.tensor_add(cT[:, cs:ce], cT[:, cs:ce],
                                                 tmp[:, :cw])
                        else:
                            nc.vector.tensor_mul(cT[:, cs:ce], sg[:, 0, :cw],
                                                 sg[:, 2, :cw])
                        th = sp.tile([128, 512], BF, tag='th')
                        nc.scalar.activation(out=th[:, :cw], in_=cT[:, cs:ce],
                                             func=AF.Tanh)
                        nc.vector.tensor_mul(hT[:, cs:ce], sg[:, 3, :cw],
                                             th[:, :cw])

            # ============ user renorm + feat + transposes + AG1 ============
            if PHASE < 3:
                raise _PhaseDone()
            with (
                tc.tile_pool(name='feat', bufs=1) as fp,
                tc.tile_pool(name='feat_ps', bufs=2, space='PSUM') as fps,
                tc.tile_pool(name='tp_ps', bufs=2, space='PSUM') as tps,
            ):
                u16 = renorm(fp, usr, JU)
                longT = fp.tile([128, NL], BF)
                for j in range(JU):
                    pt = tps.tile([128, 128], BF, tag='tp')
                    nc.tensor.transpose(pt[:], u16[:, j, :], ident[:])
                    nc.vector.tensor_copy(longT[:, j * 128:(j + 1) * 128],
                                          pt[:])

                featT = fp.tile([128, NL], BF)
                for c in range((NL + 511) // 512):
                    cs = c * 512
                    cw = min(512, NL - cs)
                    ps = fps.tile([128, 512], FT, tag='fps')
                    nc.tensor.matmul(ps[:, :cw], lhsT=w1_16[:, 0, :],
                                     rhs=longT[:, cs:cs + cw], start=True,
                                     stop=False)
                    nc.tensor.matmul(ps[:, :cw], lhsT=w1_16[:, 1, :],
                                     rhs=hT[:, cs:cs + cw], start=False,
                                     stop=True)
                    nc.scalar.activation(out=featT[:, cs:cs + cw],
                                         in_=ps[:, :cw], func=AF.Relu)

                fr = fp.tile([128, JU, 128], BF)
                hr = fp.tile([128, JU, 128], BF)
                for j in range(JU):
                    pt = tps.tile([128, 128], BF, tag='tp')
                    nc.tensor.transpose(pt[:], featT[:, j * 128:(j + 1) * 128],
                                        ident[:])
                    nc.vector.tensor_copy(fr[:, j, :], pt[:])
                    pt2 = tps.tile([128, 128], BF, tag='tp')
                    nc.tensor.transpose(pt2[:], hT[:, j * 128:(j + 1) * 128],
                                        ident[:])
                    nc.vector.tensor_copy(hr[:, j, :], pt2[:])
                nc.sync.dma_start(cont_ap(HNROWS, JU), hr[:])
                nc.sync.dma_start(cont_ap(AGIN, JU), fr[:])
                curs = fp.tile([128, 1, 128], BF)
                nc.vector.memset(curs[:], 0.0)
                nc.gpsimd.dma_gather(
                    out_ap=curs[:], in_ap=HNROWS[:],
                    idxs_ap=seg_ap('hnrow', 112),
                    num_idxs=112, num_idxs_reg=112, elem_size=D,
                    transpose=False, queue_num=0)
                ag = AGIN[:]
                nc.sync.dma_start(
                    bass.AP(tensor=ag.tensor, offset=ag.offset + NL * D,
                            ap=[[D, 128], [1, D]]),
                    curs[:, 0, :])
                nc.gpsimd.collective_compute(
                    'AllGather', OP.bypass,
                    replica_groups=[list(range(NCORES))],
                    ins=[AGIN.opt()], outs=[FTAB.opt()])

            # ================= GAT layers =================
            def gat_layer(pool, pps, tab, pay_seg, fd_seg, nch, dst_off, ndst,
                          gw16, gb_sb, res_seg, res_n):
                E = nch * 128
                pay = pool.tile([128, nch, 128], BF, tag='pay')
                gather_rows(pay, tab, pay_seg, E)
                fde = pool.tile([128, nch, 128], BF, tag='fde')
                gather_rows(fde, tab, fd_seg, E)
                score = pool.tile([128, nch], FT, tag='score')
                prod = pool.tile([128, nch, 128], BF, tag='prod')
                nc.vector.tensor_mul(prod[:], pay[:], fde[:])
                nc.vector.tensor_reduce(out=score[:], in_=prod[:],
                                        axis=mybir.AxisListType.X, op=OP.add)
                w = pool.tile([128, nch], FT, tag='w')
                nc.scalar.activation(out=w[:], in_=score[:], func=AF.Exp)
                w16 = pool.tile([128, nch], BF, tag='w16')
                nc.vector.tensor_copy(w16[:], w[:])
                wpay = pool.tile([128, nch, 128], BF, tag='wpay')
                wv = w16[:]
                bc = bass.AP(tensor=wv.tensor, offset=wv.offset,
                             ap=[wv.ap[0], wv.ap[1], [0, 128]])
                nc.vector.tensor_tensor(out=wpay[:], in0=pay[:], in1=bc,
                                        op=OP.mult)
                aggp = pps.tile([128, 512], FT, tag='aggp')
                zp = pps.tile([1, 512], FT, tag='zp')
                for c in range(nch):
                    oh = pool.tile([128, 512], BF, tag='oh', bufs=2)
                    nc.vector.tensor_scalar(
                        out=oh[:, :ndst], in0=iotaf[:, :ndst],
                        scalar1=dstf[:, dst_off + c:dst_off + c + 1],
                        scalar2=None, op0=OP.is_equal)
                    nc.tensor.matmul(aggp[:, :ndst], lhsT=wpay[:, c, :],
                                     rhs=oh[:, :ndst], start=(c == 0),
                                     stop=(c == nch - 1))
                    nc.tensor.matmul(zp[:, :ndst], lhsT=w16[:, c:c + 1],
                                     rhs=oh[:, :ndst], start=(c == 0),
                                     stop=(c == nch - 1))
                zsb = pool.tile([1, 512], FT, tag='zsb')
                nc.vector.tensor_copy(zsb[:, :ndst], zp[:, :ndst])
                zr = pool.tile([1, 512], FT, tag='zr')
                nc.vector.reciprocal(zr[:, :ndst], zsb[:, :ndst])
                rbp = pps.tile([128, 512], FT, tag='rbp')
                nc.tensor.matmul(rbp[:, :ndst], lhsT=ones1[:],
                                 rhs=zr[:, :ndst], start=True, stop=True)
                rb = pool.tile([128, 512], FT, tag='rb')
                nc.vector.tensor_copy(rb[:, :ndst], rbp[:, :ndst])
                aggn = pool.tile([128, 512], BF, tag='aggn')
                nc.vector.tensor_mul(aggn[:, :ndst], aggp[:, :ndst],
                                     rb[:, :ndst])
                rp = pps.tile([128, 512], FT, tag='rp')
                nc.tensor.matmul(rp[:, :ndst], lhsT=gw16[:],
                                 rhs=aggn[:, :ndst], start=True, stop=True)
                rl = pool.tile([128, 512], BF, tag='rl')
                nc.scalar.activation(out=rl[:, :ndst], in_=rp[:, :ndst],
                                     func=AF.Relu, bias=gb_sb[:])
                rn = _rup(res_n, 128)
                fdr = pool.tile([128, rn], BF, tag='fdr')
                gather_t(fdr[:].rearrange('p (o n) -> p o n', o=1), 0, tab,
                         res_seg, rn)
                outT = pool.tile([128, rn], BF, tag='outT')
                if rn > ndst:
                    nc.vector.memset(outT[:, ndst:], 0.0)
                nc.vector.tensor_add(outT[:, :ndst], fdr[:, :ndst],
                                     rl[:, :ndst])
                return outT

            if PHASE < 4:
                raise _PhaseDone()
            with (
                tc.tile_pool(name='gat', bufs=1) as gp0,
                tc.tile_pool(name='gat_ps', bufs=1, space='PSUM') as gps,
            ):
                f1T = gat_layer(gp0, gps, FTAB, 'g0pay', 'g0fd', NCH0, 0,
                                DST0_PER_CORE, gw0_16, gb0_sb, 'g0res', B1)
                a2 = gp0.tile([128, 3, 128], BF)
                for j in range(3):
                    pt = gps.tile([128, 128], BF, tag='tp2', bufs=2)
                    nc.tensor.transpose(pt[:], f1T[:, j * 128:(j + 1) * 128],
                                        ident[:])
                    nc.vector.tensor_copy(a2[:, j, :], pt[:])
                nc.sync.dma_start(cont_ap(AG2IN, 3), a2[:])
                nc.gpsimd.collective_compute(
                    'AllGather', OP.bypass,
                    replica_groups=[list(range(NCORES))],
                    ins=[AG2IN.opt()], outs=[F1TAB.opt()])

                f2T = gat_layer(gp0, gps, F1TAB, 'g1pay', 'g1fd', NCH1, NCH0,
                                N2, gw1_16, gb1_sb, 'g1res', 512)

                curT = gp0.tile([128, 512], BF)
                gather_t(curT[:].rearrange('p (o n) -> p o n', o=1), 0, FTAB,
                         'cur', 512)
                srp = gps.tile([128, 512], FT, tag='srp')
                nc.tensor.matmul(srp[:], lhsT=w2_16[:, 0, :], rhs=curT[:],
                                 start=True, stop=False)
                nc.tensor.matmul(srp[:], lhsT=w2_16[:, 1, :], rhs=f2T[:, :512],
                                 start=False, stop=True)
                sr16 = glob.tile([128, 512], BF)
                nc.vector.tensor_copy(sr16[:], srp[:])

            # ================= logits =================
            if PHASE < 5:
                raise _PhaseDone()
            with (
                tc.tile_pool(name='lg', bufs=1) as lp,
                tc.tile_pool(name='lg_o', bufs=4) as lop,
                tc.tile_pool(name='lg_ps', bufs=4, space='PSUM') as lps,
            ):
                ls16 = renorm(lp, ishard, JL, rows=LSH)
                nc.sync.dma_start(cont_ap(LSHARD, JL), ls16[:])
                itemT = lp.tile([128, LSHP], BF)
                gather_t(itemT[:].rearrange('p (o n) -> p o n', o=1), 0,
                         LSHARD, 'lsh', LSHP)
                for m in range(4):
                    for n in range((LSH + 511) // 512):
                        cs = n * 512
                        cw = min(512, LSH - cs)
                        ps = lps.tile([128, 512], FT, tag='lgps')
                        nc.tensor.matmul(ps[:, :cw],
                                         lhsT=sr16[:, m * 128:(m + 1) * 128],
                                         rhs=itemT[:, cs:cs + cw],
                                         start=True, stop=True)
                        ob = lop.tile([128, 512], FT, tag='ob')
                        nc.vector.tensor_copy(ob[:, :cw], ps[:, :cw])
                        nc.sync.dma_start(
                            bass.AP(tensor=out_p, offset=m * 128 * LSH + cs,
                                    ap=[[LSH, 128], [1, cw]]),
                            ob[:, :cw])

            ctx.__exit__(None, None, None)
        except _PhaseDone:
            ctx.__exit__(None, None, None)
    nc.compile()
    return nc


_CACHE = {}


def prepare(inputs):
    in_maps, meta = host_prep(inputs)
    import os
    key = (meta['NL'], meta['VC'], meta['E0C'], tuple(meta['act']),
           os.environ.get('KPHASE', '9'), os.environ.get('KSUB', '9'),
           os.environ.get('KT', '99'), os.environ.get('KCH', '9'))
    if key not in _CACHE:
        _CACHE[key] = build_program(meta)
    return _CACHE[key], in_maps, meta


def kernel(**inputs):
    from concourse.bass_utils import run_bass_kernel_spmd
    nc, in_maps, meta = prepare(inputs)
    res = run_bass_kernel_spmd(nc, in_maps, list(range(NCORES)))
    out = np.concatenate([res.results[k]['out'] for k in range(NCORES)],
                         axis=1)
    return np.ascontiguousarray(out.astype(np.float32))



# revision 33
# speedup vs baseline: 169.5006x; 169.5006x over previous
"""DGRec kernel for 8 TRN2 NeuronCores (Bass/Tile) — v2.

Strategy vs v1:
  - Host pre-renorms item/user embeddings (f32 math) and pre-gathers the
    LSTM inputs into a transposed [128, sum_t act128(t)] bf16 buffer, so the
    device LSTM is a pure matmul/activation pipeline (no gpsimd gathers, no
    on-device renorm, no ITBL/LSHARD staging round-trips).
  - Identity row mapping in the feat table (row = local session index).
  - Weights uploaded as bf16; logits item shard uploaded pre-transposed.
  - GAT gathers issued as single big dma_gather calls on separate queues
    (desc-gen is ~1us + 0.34ns/desc).
  - GAT1 edge-sharded 8x across cores, partial softmax numerator/denominator
    combined with a f32 AllReduce.
  - Logits written as fp16 (halves the output DMA), host casts to f32.
"""
import sys
sys.path.insert(0, '/opt/trn_rl_repo')

import numpy as np
import ml_dtypes

BF16 = ml_dtypes.bfloat16


class _PhaseDone(Exception):
    pass


NCORES = 8
D = 128
T = 20
N0 = 25600
N1 = 2560
N2 = 512
E0 = 25600
E1 = 5120
NI = 50000
CURB = 128          # cur block rows per core (slot CURB-1 of core0 = zero row)
DST0_PER_CORE = N1 // NCORES  # 320
B1 = 384            # f1 block rows per core (320 padded to 384)
LSH = NI // NCORES  # 6250
LSHP = 6272         # rup(LSH, 128)
LCH1 = (E1 // 128) // NCORES  # 5 edge chunks of GAT1 per core


def _rup(x, m):
    return (int(x) + m - 1) // m * m


def _wrap16(idx):
    """[n] int -> [128, n//16] int16: idx i at [i%16, i//16], replicated x8."""
    idx = np.asarray(idx, np.int16)
    n = len(idx)
    assert n % 16 == 0
    a = idx.reshape(n // 16, 16).T  # [16, n//16]
    return np.tile(a, (8, 1))


def _renorm_rows(x):
    n = np.linalg.norm(x, axis=-1, keepdims=True)
    return x * np.minimum(1.0, 1.0 / np.maximum(n, 1e-12))


def host_prep(inputs):
    lens = np.asarray(inputs['lens']).astype(np.int64)
    seqs = np.asarray(inputs['padded_seqs']).astype(np.int64)
    uids = np.asarray(inputs['uids']).astype(np.int64)
    cur_sidx = np.asarray(inputs['cur_sidx']).astype(np.int64)
    src0 = np.asarray(inputs['src0']).astype(np.int64)
    dst0 = np.asarray(inputs['dst0']).astype(np.int64)
    idx0 = np.asarray(inputs['idx0']).astype(np.int64)
    src1 = np.asarray(inputs['src1']).astype(np.int64)
    dst1 = np.asarray(inputs['dst1']).astype(np.int64)
    idx1 = np.asarray(inputs['idx1']).astype(np.int64)
    user_emb = np.asarray(inputs['user_emb'], np.float32)
    item_emb = np.asarray(inputs['item_emb'], np.float32)

    # ---- host renorm of tables (f32) -> bf16
    it = item_emb.copy()
    it[0] = 0.0
    itn16 = _renorm_rows(it).astype(BF16)          # [NI+1, D]

    # ---- live sessions, sorted by len desc, per-len-group padded to mult 8
    live_mask = np.zeros(N0, bool)
    live_mask[src0] = True
    live_mask[idx0] = True
    live_mask[cur_sidx] = True
    live = np.where(live_mask)[0]
    order = live[np.argsort(-lens[live], kind='stable')]
    lens_live = lens[order]
    parts = []
    grp_ceil = {}
    for L in range(T, 0, -1):
        grp = order[lens_live == L]
        pad = (-len(grp)) % NCORES
        parts.append(grp)
        grp_ceil[L] = (len(grp) + pad) // NCORES
        if pad:
            parts.append(np.full(pad, -1, np.int64))
    order_p = np.concatenate(parts)
    percore = len(order_p) // NCORES
    NL = _rup(percore, 128)
    extra = NL * NCORES - len(order_p)
    order_p = np.concatenate([order_p, np.full(extra, -1, np.int64)])
    core_sessions = [order_p[k::NCORES] for k in range(NCORES)]
    act = [sum(grp_ceil[L] for L in range(t + 1, T + 1)) for t in range(T)]
    act = [min(a, NL) for a in act]

    # ---- FTAB layout: per-core block [NL feat rows][CURB cur rows], identity
    BLK = NL + CURB
    ZROW = NL + CURB - 1  # core 0 block, cur slot CURB-1: reserved zero row
    sess2pos = np.full(N0, -1, np.int64)
    core_local = []
    for k in range(NCORES):
        sess = core_sessions[k]
        real = sess >= 0
        li = np.where(real)[0]
        sess2pos[sess[real]] = k * BLK + li
        core_local.append({int(s): i for i, s in enumerate(sess) if s >= 0})
    # cur block: per core, unique local cur sessions -> slots 0..cnt-1
    cur_pos = {}
    cur_slot_local = [[] for _ in range(NCORES)]
    for s in np.unique(cur_sidx):
        owner = -1
        for k in range(NCORES):
            if int(s) in core_local[k]:
                owner = k
                break
        assert owner >= 0
        slot = len(cur_slot_local[owner])
        assert slot < 112
        cur_pos[int(s)] = owner * BLK + NL + slot
        cur_slot_local[owner].append(core_local[owner][int(s)])
    for s in np.unique(cur_sidx):
        sess2pos[s] = cur_pos[int(s)]

    # ---- x step offsets (shared schedule all cores)
    offs = []
    off = 0
    for t in range(T):
        offs.append(off)
        if act[t] > 0:
            off += _rup(act[t], 128)
    CTOT = off

    # ---- GAT0: edges sharded by dst range, sorted by src-OWNER core so the
    # AllToAll delivers pay rows in edge order (linear copies, no gather)
    owner_of = sess2pos // BLK          # global session -> owner core
    lrow_of = sess2pos - owner_of * BLK  # local AGIN row on owner
    g0_eord = []     # per core: edge ids sorted by src owner
    g0_cnt = np.zeros((NCORES, NCORES), np.int64)   # [k][j] = #edges
    fd_ord = []      # per core: dst order sorted by fd owner
    fd_cnt = np.zeros((NCORES, NCORES), np.int64)
    fd_rank = []     # per core: dst -> rank within its owner group
    fd_owner_l = []
    for k in range(NCORES):
        lo, hi = k * DST0_PER_CORE, (k + 1) * DST0_PER_CORE
        e = np.where((dst0 >= lo) & (dst0 < hi))[0]
        eo = owner_of[src0[e]]
        srt = np.argsort(eo, kind='stable')
        e = e[srt]
        g0_eord.append(e)
        for j in range(NCORES):
            g0_cnt[k][j] = int((owner_of[src0[e]] == j).sum())
        fo = owner_of[idx0[lo:hi]]                  # [320] fd owner per dst
        dsrt = np.argsort(fo, kind='stable')        # dst order by owner
        fd_ord.append(dsrt)
        rank = np.zeros(DST0_PER_CORE, np.int64)
        # rank within owner group, in dsrt order
        pos_in_grp = np.zeros(DST0_PER_CORE, np.int64)
        cnts = {}
        for i, d in enumerate(dsrt):
            o = int(fo[d])
            pos_in_grp[d] = cnts.get(o, 0)
            cnts[o] = cnts.get(o, 0) + 1
        rank = pos_in_grp
        fd_rank.append(rank)
        fd_owner_l.append(fo)
        for j in range(NCORES):
            fd_cnt[k][j] = int((fo == j).sum())
    CPAY = _rup(int(g0_cnt.max()), 128)
    CFD = _rup(max(int(fd_cnt.max()), 1), 128)
    P_A2A = CPAY + CFD + CURB
    E0C = NCORES * CPAY
    NCH0 = E0C // 128

    def f1pos(node):
        node = np.asarray(node)
        return (node // DST0_PER_CORE) * B1 + node % DST0_PER_CORE

    g1_res = f1pos(idx1)  # [512]
    # cur rows in A2AOUT: owner o's cur block at o*P_A2A + CPAY + CFD
    cur_idx = np.array(
        [(cur_pos[int(s)] // BLK) * P_A2A + CPAY + CFD
         + (cur_pos[int(s)] - (cur_pos[int(s)] // BLK) * BLK - NL)
         for s in cur_sidx], np.int64)

    meta = dict(NL=NL, BLK=BLK, E0C=E0C, NCH0=NCH0, act=act,
                offs=offs, CTOT=CTOT, CPAY=CPAY, CFD=CFD, P_A2A=P_A2A)

    # ---- IDX16 segment layout (same offsets on all cores)
    seg_off = {}
    _w = [0]

    def add_seg(name, n):
        seg_off[name] = _w[0]
        _w[0] += n // 16

    add_seg('a2asend', NCORES * P_A2A)
    add_seg('g0fd', E0C)
    add_seg('g0res', B1)
    add_seg('g1pay', LCH1 * 128)
    add_seg('g1fd', LCH1 * 128)
    add_seg('g1res', 512)
    add_seg('cur', 512)
    add_seg('hnrow', 112)
    W16 = _w[0]
    meta['seg_off'] = seg_off
    meta['W16'] = W16

    def _gate_reord(wT):
        # torch gate order [i, f, g, o] -> [i, f, o, g] so the 3 sigmoids
        # are contiguous for one fused ACT
        return np.concatenate(
            [wT[:, 0:256], wT[:, 384:512], wT[:, 256:384]], axis=1)

    wih16 = np.ascontiguousarray(_gate_reord(
        np.asarray(inputs['Wih'], np.float32).T)).astype(BF16)
    whh16 = np.ascontiguousarray(_gate_reord(
        np.asarray(inputs['Whh'], np.float32).T)).astype(BF16)
    bih_r = _gate_reord(np.asarray(inputs['bih'], np.float32)[None, :])[0]
    bhh_r = _gate_reord(np.asarray(inputs['bhh'], np.float32)[None, :])[0]
    w1t16 = np.ascontiguousarray(
        np.asarray(inputs['W1'], np.float32).T).astype(BF16)
    w2t16 = np.ascontiguousarray(
        np.asarray(inputs['W2'], np.float32).T).astype(BF16)
    gw0t16 = np.ascontiguousarray(
        np.asarray(inputs['gW0'], np.float32).T).astype(BF16)
    gw1t16 = np.ascontiguousarray(
        np.asarray(inputs['gW1'], np.float32).T).astype(BF16)

    in_maps = []
    for k in range(NCORES):
        sess = core_sessions[k]
        smax = np.maximum(sess, 0)

        # x inputs: pre-renormed, pre-gathered, transposed, step-major
        xall = np.zeros((128, CTOT), BF16)
        for t in range(T):
            a = act[t]
            if a == 0:
                break
            toks = np.where(sess[:a] >= 0, seqs[smax[:a], t], 0)
            xall[:, offs[t]:offs[t] + a] = itn16[toks].T

        # long-term user emb: renormed + transposed
        usr = _renorm_rows(user_emb[uids[smax]])
        usr[sess < 0] = 0.0
        usrT = np.ascontiguousarray(usr.T).astype(BF16)  # [128, NL]

        # logits item shard: renormed + transposed
        itemT = np.zeros((128, LSHP), BF16)
        itemT[:, :LSH] = itn16[1 + k * LSH: 1 + (k + 1) * LSH].T

        idx16 = np.zeros((128, W16), np.int16)

        def put(name, vals):
            v = np.asarray(vals, np.int64)
            assert v.min() >= -1 and v.max() < 32767, (name, v.min(), v.max())
            o = seg_off[name]
            w = _wrap16(v.astype(np.int16))
            idx16[:, o:o + w.shape[1]] = w

        # a2asend: per dest core j: [pay rows j needs from me (CPAY)]
        # [fd rows j needs (CFD)] [my cur block (CURB)]
        snd = np.zeros(NCORES * P_A2A, np.int64)
        for j in range(NCORES):
            base = j * P_A2A
            ej = g0_eord[j]
            mine = ej[owner_of[src0[ej]] == k]
            snd[base:base + len(mine)] = lrow_of[src0[mine]]
            fo = fd_owner_l[j]
            dmine = np.where(fo == k)[0]
            # order by rank (dsrt order) — fd_rank gives rank within group
            dmine = dmine[np.argsort(fd_rank[j][dmine], kind='stable')]
            fdp = idx0[j * DST0_PER_CORE + dmine]
            snd[base + CPAY:base + CPAY + len(dmine)] = lrow_of[fdp]
            snd[base + CPAY + CFD:base + P_A2A] = NL + np.arange(CURB)
        put('a2asend', snd)

        # receive-side per-edge indices into A2AOUT
        e = g0_eord[k]
        eo = owner_of[src0[e]]
        fde = np.zeros(E0C, np.int64)
        dstl = np.full(E0C, -1, np.int64)
        # edge slot: owner block j occupies [j*CPAY, j*CPAY+cnt)
        pos = np.zeros(len(e), np.int64)
        for j in range(NCORES):
            m = np.where(eo == j)[0]
            pos[m] = j * CPAY + np.arange(len(m))
        dl = dst0[e] - k * DST0_PER_CORE
        dstl[pos] = dl
        jd = fd_owner_l[k][dl]          # fd owner per edge
        rd = fd_rank[k][dl]
        fde[pos] = jd * P_A2A + CPAY + rd
        put('g0fd', fde)
        res0 = np.zeros(B1, np.int64)
        res0[:DST0_PER_CORE] = (
            fd_owner_l[k] * P_A2A + CPAY + fd_rank[k])
        put('g0res', res0)

        # GAT1 shard: this core's edge chunks
        e_lo, e_hi = k * LCH1 * 128, (k + 1) * LCH1 * 128
        put('g1pay', f1pos(src1[e_lo:e_hi]))
        put('g1fd', f1pos(idx1[dst1[e_lo:e_hi]]))
        put('g1res', g1_res)
        put('cur', cur_idx)
        hnrow = np.zeros(112, np.int64)
        cs = np.asarray(cur_slot_local[k], np.int64)
        assert len(cs) <= 112
        if len(cs):
            hnrow[:len(cs)] = cs
        put('hnrow', hnrow)

        # dst values for one-hot compare: [128, NCH0 + LCH1]
        dstl_all = np.full((128, NCH0 + LCH1), -1, np.int32)
        dstl_all[:, :NCH0] = dstl.reshape(NCH0, 128).T
        dstl_all[:, NCH0:] = dst1[e_lo:e_hi].reshape(LCH1, 128).T

        in_maps.append({
            'xall': xall,
            'usrT': usrT,
            'itemT': itemT,
            'WihT': wih16,
            'WhhT': whh16,
            'bih': bih_r,
            'bhh': bhh_r,
            'W1T': w1t16,
            'W2T': w2t16,
            'gW0T': gw0t16,
            'gW1T': gw1t16,
            'gb0': np.asarray(inputs['gb0'], np.float32),
            'gb1': np.asarray(inputs['gb1'], np.float32),
            'idx16': idx16,
            'dstl': dstl_all,
        })
    return in_maps, meta


# ============================ device program ============================

def build_program(meta, queue_map=None):
    import os
    PHASE = int(os.environ.get('KPHASE', '9'))
    import contextlib
    import concourse.bass as bass
    import concourse.mybir as mybir
    import concourse.tile as tile
    from concourse import bacc
    from concourse.masks import make_identity

    NL = meta['NL']
    BLK = meta['BLK']
    E0C = meta['E0C']
    NCH0 = meta['NCH0']
    act = meta['act']
    offs = meta['offs']
    CTOT = meta['CTOT']
    CPAY = meta['CPAY']
    CFD = meta['CFD']
    P_A2A = meta['P_A2A']
    seg = meta['seg_off']
    W16 = meta['W16']
    JU = NL // 128
    FT = mybir.dt.float32
    BF = mybir.dt.bfloat16
    F16 = mybir.dt.float16
    AF = mybir.ActivationFunctionType
    OP = mybir.AluOpType

    NQ = int(os.environ.get('KQ', '4'))
    GMAX = int(os.environ.get('KGMAX', '99999'))
    nc = bacc.Bacc("TRN2", target_bir_lowering=False, debug=False,
                   num_devices=NCORES, num_swdge_queues=NQ)

    # Tile assigns SWDGE DMA sem lanes round-robin (8 lanes) over Pool-engine
    # DMA instructions in program order, ignoring queue_num. Each lane must
    # only ever see one queue, so queue_num is driven by a queue_map recorded
    # from a first compile pass (lane % NQ), default all-0.
    _swq = [0]
    _gather_insts = []

    def gq():
        i = _swq[0]
        _swq[0] += 1
        return queue_map[i] if queue_map is not None and i < len(queue_map) \
            else 0

    def param(name, shape, dt=FT):
        return nc.declare_dram_parameter(name, list(shape), dt, isOutput=False)

    xall_p = param('xall', [128, CTOT], BF)
    usrT_p = param('usrT', [128, NL], BF)
    itemT_p = param('itemT', [128, LSHP], BF)
    WihT = param('WihT', [D, 512], BF)
    WhhT = param('WhhT', [D, 512], BF)
    bih = param('bih', [512])
    bhh = param('bhh', [512])
    W1T = param('W1T', [256, D], BF)
    W2T = param('W2T', [256, D], BF)
    gW0T = param('gW0T', [D, D], BF)
    gW1T = param('gW1T', [D, D], BF)
    gb0 = param('gb0', [D])
    gb1 = param('gb1', [D])
    idx16_p = param('idx16', [128, W16], mybir.dt.int16)
    dstl_p = param('dstl', [128, NCH0 + LCH1], mybir.dt.int32)
    out_p = nc.declare_dram_parameter('out', [N2, LSH], F16, isOutput=True)

    def rows_ap(handle_ap, j_count, base_elem=0):
        """view rows [128*j_count, D] of a DRAM tensor as [128, j, D],
        slot (p, j) <-> DRAM row 128*j + p"""
        t = handle_ap if isinstance(handle_ap, bass.AP) else handle_ap[:]
        return bass.AP(tensor=t.tensor, offset=t.offset + base_elem,
                       ap=[[D, 128], [128 * D, j_count], [1, D]])

    with tile.TileContext(nc) as tc:
        try:
            ctx = contextlib.ExitStack()
            ctx.__enter__()
            glob = ctx.enter_context(tc.tile_pool(name='glob', bufs=1))
            dram = ctx.enter_context(tc.tile_pool(name='dram', bufs=1,
                                                  space='DRAM'))

            HNROWS = dram.tile([NL, D], BF)
            AGIN = dram.tile([BLK, D], BF)
            A2AIN = dram.tile([NCORES * P_A2A, D], BF)
            A2AOUT = dram.tile([NCORES * P_A2A, D], BF)
            AG2IN = dram.tile([B1, D], BF)
            F1TAB = dram.tile([NCORES * B1, D], BF, addr_space='Shared')
            ARIN = dram.tile([129, 512], FT)
            AROUT = dram.tile([129, 512], FT)

            # ---- global constants / index tiles
            idx_sb = glob.tile([128, W16], mybir.dt.int16)
            nc.sync.dma_start(idx_sb[:], idx16_p[:])

            def seg_ap(name, n, off=0):
                o = seg[name] + off // 16
                return idx_sb[:, o:o + n // 16]

            def gather_big(out_ap3, tab, name, n, transpose, queue=None):
                """dma_gather of n idxs split into <=GMAX chunks.
                out_ap3: [128, n//128, 128] (rows) or [128, 1, n] (transp)."""
                for o in range(0, n, GMAX):
                    w = min(GMAX, n - o)
                    if transpose:
                        oap = out_ap3[:, :, o:o + w]
                    else:
                        oap = out_ap3[:, o // 128:(o + w) // 128, :]
                    _gather_insts.append(nc.gpsimd.dma_gather(
                        out_ap=oap, in_ap=tab[:], idxs_ap=seg_ap(name, w, o),
                        num_idxs=w, num_idxs_reg=w, elem_size=D,
                        transpose=transpose, queue_num=gq()))

            ident = glob.tile([128, 128], BF)
            make_identity(nc, ident[:])
            iota_i = glob.tile([128, 512], mybir.dt.int32)
            nc.gpsimd.iota(iota_i[:], pattern=[[1, 512]], base=0,
                           channel_multiplier=0)
            iotaf = glob.tile([128, 512], FT)
            nc.vector.tensor_copy(iotaf[:], iota_i[:])
            dstl_i = glob.tile([128, NCH0 + LCH1], mybir.dt.int32)
            nc.sync.dma_start(dstl_i[:], dstl_p[:])
            dstf = glob.tile([128, NCH0 + LCH1], FT)
            nc.vector.tensor_copy(dstf[:], dstl_i[:])
            ones1 = glob.tile([1, 128], FT)
            nc.vector.memset(ones1[:], 1.0)

            # ---- weights (already bf16)
            wih16 = glob.tile([128, 512], BF)
            nc.sync.dma_start(wih16[:], WihT[:])
            whh16 = glob.tile([128, 512], BF)
            nc.sync.dma_start(whh16[:], WhhT[:])
            w1_16 = glob.tile([128, 2, 128], BF)
            nc.sync.dma_start(w1_16[:], rows_ap(W1T, 2))
            w2_16 = glob.tile([128, 2, 128], BF)
            nc.sync.dma_start(w2_16[:], rows_ap(W2T, 2))
            gw0_16 = glob.tile([128, 128], BF)
            nc.sync.dma_start(gw0_16[:], gW0T[:])
            gw1_16 = glob.tile([128, 128], BF)
            nc.sync.dma_start(gw1_16[:], gW1T[:])
            gb0_sb = glob.tile([128, 1], FT)
            nc.sync.dma_start(gb0_sb[:], bass.AP(tensor=gb0, offset=0,
                                                 ap=[[1, 128], [1, 1]]))
            gb1_sb = glob.tile([128, 1], FT)
            nc.sync.dma_start(gb1_sb[:], bass.AP(tensor=gb1, offset=0,
                                                 ap=[[1, 128], [1, 1]]))
            bi_sb = glob.tile([128, 4], FT)
            nc.sync.dma_start(bi_sb[:], bass.AP(tensor=bih, offset=0,
                                                ap=[[1, 128], [128, 4]]))
            bh_sb = glob.tile([128, 4], FT)
            nc.sync.dma_start(bh_sb[:], bass.AP(tensor=bhh, offset=0,
                                                ap=[[1, 128], [128, 4]]))
            bias = glob.tile([128, 4], FT)
            nc.vector.tensor_add(bias[:], bi_sb[:], bh_sb[:])

            # ---- big resident inputs
            usrT_sb = glob.tile([128, NL], BF)
            nc.sync.dma_start(usrT_sb[:], usrT_p[:])
            itemT_sb = glob.tile([128, LSHP], BF)
            nc.sync.dma_start(itemT_sb[:], itemT_p[:])
            xall_sb = glob.tile([128, CTOT], BF)
            # split the big x load so step 0 can start early
            x0w = offs[1] if T > 1 else CTOT
            nc.sync.dma_start(xall_sb[:, :x0w], xall_p[:, :x0w])
            nc.sync.dma_start(xall_sb[:, x0w:], xall_p[:, x0w:])

            # ================= LSTM =================
            if PHASE < 2:
                raise _PhaseDone()
            hT = glob.tile([128, NL], BF)
            cT = glob.tile([128, NL], BF)
            nc.vector.memset(hT[:], 0.0)
            nc.vector.memset(cT[:], 0.0)

            with (
                tc.tile_pool(name='lstm_g', bufs=3) as sp,
                tc.tile_pool(name='lstm_ps', bufs=2, space='PSUM') as gp,
            ):
                for t in range(T):
                    a = act[t]
                    if a == 0:
                        break
                    nch = (a + 511) // 512
                    for c in range(nch):
                        cs = c * 512
                        cw = min(512, a - cs)
                        ce = cs + cw
                        xs = offs[t] + cs
                        g4 = gp.tile([128, 4, 512], FT, tag='g4')
                        for g in range(4):
                            nc.tensor.matmul(
                                g4[:, g, :cw],
                                lhsT=wih16[:, g * 128:(g + 1) * 128],
                                rhs=xall_sb[:, xs:xs + cw],
                                start=True, stop=(t == 0))
                            if t > 0:
                                nc.tensor.matmul(
                                    g4[:, g, :cw],
                                    lhsT=whh16[:, g * 128:(g + 1) * 128],
                                    rhs=hT[:, cs:ce], start=False, stop=True)
                        # gates reordered [i, f, o, g]: one DVE bias-add +
                        # one fused sigmoid ACT over the 3 sigmoid gates
                        g3b = sp.tile([128, 3, 512], BF, tag='g3b')
                        bv = bias[:]
                        b3 = bass.AP(tensor=bv.tensor, offset=bv.offset,
                                     ap=[bv.ap[0], [1, 3], [0, cw]])
                        nc.vector.tensor_tensor(out=g3b[:, :, :cw],
                                                in0=g4[:, 0:3, :cw], in1=b3,
                                                op=OP.add)
                        sg = sp.tile([128, 3, 512], BF, tag='sg')
                        nc.scalar.activation(out=sg[:, :, :cw],
                                             in_=g3b[:, :, :cw],
                                             func=AF.Sigmoid)
                        tg = sp.tile([128, 512], BF, tag='tg')
                        nc.scalar.activation(out=tg[:, :cw],
                                             in_=g4[:, 3, :cw], func=AF.Tanh,
                                             bias=bias[:, 3:4])
                        if t > 0:
                            tmp = sp.tile([128, 512], BF, tag='tmp')
                            nc.vector.tensor_mul(tmp[:, :cw], sg[:, 0, :cw],
                                                 tg[:, :cw])
                            nc.vector.tensor_mul(cT[:, cs:ce], cT[:, cs:ce],
                                                 sg[:, 1, :cw])
                            nc.vector.tensor_add(cT[:, cs:ce], cT[:, cs:ce],
                                                 tmp[:, :cw])
                        else:
                            nc.vector.tensor_mul(cT[:, cs:ce], sg[:, 0, :cw],
                                                 tg[:, :cw])
                        th = sp.tile([128, 512], BF, tag='th')
                        nc.scalar.activation(out=th[:, :cw], in_=cT[:, cs:ce],
                                             func=AF.Tanh)
                        nc.vector.tensor_mul(hT[:, cs:ce], sg[:, 2, :cw],
                                             th[:, :cw])

            # ============ feat + transposes + AG1 ============
            if PHASE < 3:
                raise _PhaseDone()
            with (
                tc.tile_pool(name='feat', bufs=1) as fp,
                tc.tile_pool(name='feat_ps', bufs=2, space='PSUM') as fps,
                tc.tile_pool(name='tp_ps', bufs=2, space='PSUM') as tps,
            ):
                featT = fp.tile([128, NL], BF)
                for c in range((NL + 511) // 512):
                    cs = c * 512
                    cw = min(512, NL - cs)
                    ps = fps.tile([128, 512], FT, tag='fps')
                    nc.tensor.matmul(ps[:, :cw], lhsT=w1_16[:, 0, :],
                                     rhs=usrT_sb[:, cs:cs + cw], start=True,
                                     stop=False)
                    nc.tensor.matmul(ps[:, :cw], lhsT=w1_16[:, 1, :],
                                     rhs=hT[:, cs:cs + cw], start=False,
                                     stop=True)
                    nc.scalar.activation(out=featT[:, cs:cs + cw],
                                         in_=ps[:, :cw], func=AF.Relu)

                fr = fp.tile([128, JU, 128], BF)
                hr = fp.tile([128, JU, 128], BF)
                for j in range(JU):
                    pt = tps.tile([128, 128], BF, tag='tp')
                    nc.tensor.transpose(pt[:], featT[:, j * 128:(j + 1) * 128],
                                        ident[:])
                    nc.vector.tensor_copy(fr[:, j, :], pt[:])
                    pt2 = tps.tile([128, 128], BF, tag='tp')
                    nc.tensor.transpose(pt2[:], hT[:, j * 128:(j + 1) * 128],
                                        ident[:])
                    nc.vector.tensor_copy(hr[:, j, :], pt2[:])
                nc.sync.dma_start(rows_ap(HNROWS, JU), hr[:])
                nc.sync.dma_start(rows_ap(AGIN, JU), fr[:])
                curs = fp.tile([128, 1, 128], BF)
                nc.vector.memset(curs[:], 0.0)
                _gather_insts.append(nc.gpsimd.dma_gather(
                    out_ap=curs[:], in_ap=HNROWS[:],
                    idxs_ap=seg_ap('hnrow', 112),
                    num_idxs=112, num_idxs_reg=112, elem_size=D,
                    transpose=False, queue_num=gq()))
                ag = AGIN[:]
                nc.sync.dma_start(
                    bass.AP(tensor=ag.tensor, offset=ag.offset + NL * D,
                            ap=[[D, 128], [1, D]]),
                    curs[:, 0, :])
                # assemble AllToAll send slices by gathering from AGIN
                NP8 = NCORES * P_A2A
                snd = fp.tile([128, NP8 // 128, 128], BF)
                gather_big(snd[:], AGIN, 'a2asend', NP8, False, 0)
                nc.sync.dma_start(rows_ap(A2AIN, NP8 // 128), snd[:])
                nc.gpsimd.collective_compute(
                    'AllToAll', OP.bypass,
                    replica_groups=[list(range(NCORES))],
                    ins=[A2AIN.opt()], outs=[A2AOUT.opt()])

            # ================= GAT layers =================
            def gat_edges(pool, pps, tab, pay_a2a, pay_seg, fd_seg, nch,
                          dst_off, ndst, aggp, zp, q0=1):
                """edge fetch + edge softmax numerator/denominator matmuls
                into PSUM aggp [128, ndst] / zp [1, ndst].
                pay_a2a=True: pay rows arrive in edge order in A2AOUT's
                per-owner slices -> 8 linear DMAs. Else gather via pay_seg."""
                E = nch * 128
                pay = pool.tile([128, nch, 128], BF, tag='pay')
                if pay_a2a:
                    t = tab[:]
                    jc = CPAY // 128
                    for j in range(NCORES):
                        nc.sync.dma_start(
                            pay[:, j * jc:(j + 1) * jc, :],
                            bass.AP(tensor=t.tensor,
                                    offset=t.offset + j * P_A2A * D,
                                    ap=[[D, 128], [128 * D, jc], [1, D]]))
                else:
                    gather_big(pay[:], tab, pay_seg, E, False, q0)
                fde = pool.tile([128, nch, 128], BF, tag='fde')
                gather_big(fde[:], tab, fd_seg, E, False, q0 + 1)
                score = pool.tile([128, nch], FT, tag='score')
                prod = pool.tile([128, nch, 128], BF, tag='prod')
                nc.vector.tensor_mul(prod[:], pay[:], fde[:])
                nc.vector.tensor_reduce(out=score[:], in_=prod[:],
                                        axis=mybir.AxisListType.X, op=OP.add)
                w = pool.tile([128, nch], FT, tag='w')
                nc.scalar.activation(out=w[:], in_=score[:], func=AF.Exp)
                w16 = pool.tile([128, nch], BF, tag='w16')
                nc.vector.tensor_copy(w16[:], w[:])
                wpay = pool.tile([128, nch, 128], BF, tag='wpay')
                wv = w16[:]
                bc = bass.AP(tensor=wv.tensor, offset=wv.offset,
                             ap=[wv.ap[0], wv.ap[1], [0, 128]])
                nc.vector.tensor_tensor(out=wpay[:], in0=pay[:], in1=bc,
                                        op=OP.mult)
                # one-hot built 4 chunks per DVE op
                for cg in range(0, nch, 4):
                    gn = min(4, nch - cg)
                    ohg = pool.tile([128, 4, 512], BF, tag='oh', bufs=2)
                    io = iotaf[:]
                    io_b = bass.AP(tensor=io.tensor, offset=io.offset,
                                   ap=[io.ap[0], [0, gn], [1, ndst]])
                    dv = dstf[:, dst_off + cg:dst_off + cg + gn]
                    dv_b = bass.AP(tensor=dv.tensor, offset=dv.offset,
                                   ap=[dv.ap[0], [1, gn], [0, ndst]])
                    nc.vector.tensor_tensor(out=ohg[:, :gn, :ndst],
                                            in0=io_b, in1=dv_b,
                                            op=OP.is_equal)
                    for ci in range(gn):
                        c = cg + ci
                        nc.tensor.matmul(aggp[:, :ndst],
                                         lhsT=wpay[:, c, :],
                                         rhs=ohg[:, ci, :ndst],
                                         start=(c == 0),
                                         stop=(c == nch - 1))
                        nc.tensor.matmul(zp[:, :ndst], lhsT=w16[:, c:c + 1],
                                         rhs=ohg[:, ci, :ndst],
                                         start=(c == 0),
                                         stop=(c == nch - 1))

            def gat_norm_out(pool, pps, tab, aggsb, zsb, ndst, gw16, gb_sb,
                             res_seg, res_n, q0=3):
                """agg/z -> normalized -> gw matmul -> relu -> +residual"""
                zr = pool.tile([1, 512], FT, tag='zr')
                nc.vector.reciprocal(zr[:, :ndst], zsb[:, :ndst])
                rbp = pps.tile([128, 512], FT, tag='rbp')
                nc.tensor.matmul(rbp[:, :ndst], lhsT=ones1[:],
                                 rhs=zr[:, :ndst], start=True, stop=True)
                rb = pool.tile([128, 512], FT, tag='rb')
                nc.vector.tensor_copy(rb[:, :ndst], rbp[:, :ndst])
                aggn = pool.tile([128, 512], BF, tag='aggn')
                nc.vector.tensor_mul(aggn[:, :ndst], aggsb[:, :ndst],
                                     rb[:, :ndst])
                rp = pps.tile([128, 512], FT, tag='rp')
                nc.tensor.matmul(rp[:, :ndst], lhsT=gw16[:],
                                 rhs=aggn[:, :ndst], start=True, stop=True)
                rl = pool.tile([128, 512], BF, tag='rl')
                nc.scalar.activation(out=rl[:, :ndst], in_=rp[:, :ndst],
                                     func=AF.Relu, bias=gb_sb[:])
                rn = _rup(res_n, 128)
                fdr = pool.tile([128, rn], BF, tag='fdr')
                gather_big(fdr[:].rearrange('p (o n) -> p o n', o=1),
                           tab, res_seg, rn, True, q0)
                outT = pool.tile([128, rn], BF, tag='outT')
                if rn > ndst:
                    nc.vector.memset(outT[:, ndst:], 0.0)
                nc.vector.tensor_add(outT[:, :ndst], fdr[:, :ndst],
                                     rl[:, :ndst])
                return outT

            if PHASE < 4:
                raise _PhaseDone()
            with (
                tc.tile_pool(name='gat', bufs=1) as gp0,
                tc.tile_pool(name='gat_ps', bufs=1, space='PSUM') as gps,
            ):
                # ---- GAT0 (dst-sharded)
                aggp = gps.tile([128, 512], FT, tag='aggp')
                zp = gps.tile([1, 512], FT, tag='zp')
                gat_edges(gp0, gps, A2AOUT, True, None, 'g0fd', NCH0, 0,
                          DST0_PER_CORE, aggp, zp, q0=1)
                aggsb = gp0.tile([128, 512], FT, tag='aggsb')
                nc.vector.tensor_copy(aggsb[:, :DST0_PER_CORE],
                                      aggp[:, :DST0_PER_CORE])
                zsb = gp0.tile([1, 512], FT, tag='zsb')
                nc.vector.tensor_copy(zsb[:, :DST0_PER_CORE],
                                      zp[:, :DST0_PER_CORE])
                f1T = gat_norm_out(gp0, gps, A2AOUT, aggsb, zsb,
                                   DST0_PER_CORE, gw0_16, gb0_sb, 'g0res',
                                   B1, q0=3)
                a2 = gp0.tile([128, 3, 128], BF)
                for j in range(3):
                    pt = gps.tile([128, 128], BF, tag='tp2', bufs=2)
                    nc.tensor.transpose(pt[:], f1T[:, j * 128:(j + 1) * 128],
                                        ident[:])
                    nc.vector.tensor_copy(a2[:, j, :], pt[:])
                nc.sync.dma_start(rows_ap(AG2IN, 3), a2[:])
                nc.gpsimd.collective_compute(
                    'AllGather', OP.bypass,
                    replica_groups=[list(range(NCORES))],
                    ins=[AG2IN.opt()], outs=[F1TAB.opt()])

                # ---- GAT1 (edge-sharded, AllReduce partials)
                aggp1 = gps.tile([128, 512], FT, tag='aggp')
                zp1 = gps.tile([1, 512], FT, tag='zp')
                gat_edges(gp0, gps, F1TAB, False, 'g1pay', 'g1fd', LCH1,
                          NCH0, N2, aggp1, zp1, q0=1)
                agg1sb = gp0.tile([128, 512], FT, tag='agg1sb')
                nc.vector.tensor_copy(agg1sb[:], aggp1[:])
                z1sb = gp0.tile([1, 512], FT, tag='z1sb')
                nc.vector.tensor_copy(z1sb[:], zp1[:])
                ar = ARIN[:]
                nc.sync.dma_start(
                    bass.AP(tensor=ar.tensor, offset=ar.offset,
                            ap=[[512, 128], [1, 512]]), agg1sb[:])
                nc.sync.dma_start(
                    bass.AP(tensor=ar.tensor, offset=ar.offset + 128 * 512,
                            ap=[[512, 1], [1, 512]]), z1sb[:])
                nc.gpsimd.collective_compute(
                    'AllReduce', OP.add,
                    replica_groups=[list(range(NCORES))],
                    ins=[ARIN.opt()], outs=[AROUT.opt()])
                aro = AROUT[:]
                agg1r = gp0.tile([128, 512], FT, tag='agg1r')
                nc.sync.dma_start(
                    agg1r[:],
                    bass.AP(tensor=aro.tensor, offset=aro.offset,
                            ap=[[512, 128], [1, 512]]))
                z1r = gp0.tile([1, 512], FT, tag='z1r')
                nc.sync.dma_start(
                    z1r[:],
                    bass.AP(tensor=aro.tensor, offset=aro.offset + 128 * 512,
                            ap=[[512, 1], [1, 512]]))
                f2T = gat_norm_out(gp0, gps, F1TAB, agg1r, z1r, N2,
                                   gw1_16, gb1_sb, 'g1res', 512, q0=3)

                # ---- cur + sr
                curT = gp0.tile([128, 512], BF)
                gather_big(curT[:].rearrange('p (o n) -> p o n', o=1),
                           A2AOUT, 'cur', 512, True, 0)
                srp = gps.tile([128, 512], FT, tag='srp')
                nc.tensor.matmul(srp[:], lhsT=w2_16[:, 0, :], rhs=curT[:],
                                 start=True, stop=False)
                nc.tensor.matmul(srp[:], lhsT=w2_16[:, 1, :], rhs=f2T[:, :512],
                                 start=False, stop=True)
                sr16 = glob.tile([128, 512], BF)
                nc.vector.tensor_copy(sr16[:], srp[:])

            # ================= logits =================
            if PHASE < 5:
                raise _PhaseDone()
            with (
                tc.tile_pool(name='lg_o', bufs=4) as lop,
                tc.tile_pool(name='lg_ps', bufs=4, space='PSUM') as lps,
            ):
                ncopy = 0
                for m in range(4):
                    for n in range((LSH + 511) // 512):
                        cs = n * 512
                        cw = min(512, LSH - cs)
                        ps = lps.tile([128, 512], FT, tag='lgps')
                        nc.tensor.matmul(ps[:, :cw],
                                         lhsT=sr16[:, m * 128:(m + 1) * 128],
                                         rhs=itemT_sb[:, cs:cs + cw],
                                         start=True, stop=True)
                        ob = lop.tile([128, 512], F16, tag='ob')
                        if ncopy % 2 == 0:
                            nc.vector.tensor_copy(ob[:, :cw], ps[:, :cw])
                        else:
                            nc.scalar.activation(out=ob[:, :cw],
                                                 in_=ps[:, :cw], func=AF.Copy)
                        ncopy += 1
                        nc.sync.dma_start(
                            bass.AP(tensor=out_p, offset=m * 128 * LSH + cs,
                                    ap=[[LSH, 128], [1, cw]]),
                            ob[:, :cw])

            ctx.__exit__(None, None, None)
        except _PhaseDone:
            ctx.__exit__(None, None, None)
    nc.compile()
    # record each gather's Tile-assigned DMASW lane (11..18 = lanes 0..7)
    lanes = []
    for bi in _gather_insts:
        proc = getattr(bi.ins, 'bass_scheduled_proc', None)
        lanes.append((proc - 11) % 8 if proc is not None and proc >= 11
                     and proc <= 18 else 0)
    nc._gather_lanes = lanes
    return nc


_CACHE = {}


def prepare(inputs):
    in_maps, meta = host_prep(inputs)
    import os
    key = (meta['NL'], meta['E0C'], meta['CTOT'], tuple(meta['act']),
           os.environ.get('KPHASE', '9'), os.environ.get('KQ', '4'),
           os.environ.get('KGMAX', '99999'))
    if key not in _CACHE:
        nc1 = build_program(meta)
        NQ = int(os.environ.get('KQ', '4'))
        qmap = [ln % NQ for ln in nc1._gather_lanes]
        if any(q != 0 for q in qmap):
            _CACHE[key] = build_program(meta, queue_map=qmap)
        else:
            _CACHE[key] = nc1
    return _CACHE[key], in_maps, meta


def kernel(**inputs):
    from concourse.bass_utils import run_bass_kernel_spmd
    nc, in_maps, meta = prepare(inputs)
    res = run_bass_kernel_spmd(nc, in_maps, list(range(NCORES)))
    out = np.concatenate([res.results[k]['out'] for k in range(NCORES)],
                         axis=1)
    return np.ascontiguousarray(out.astype(np.float32))
